# revision 11
# baseline (speedup 1.0000x reference)
"""Bass/Trainium2 kernel for nn_DynamicMoELayer (moe_routing).

Strategy: shard tokens (T=8192) across 8 NeuronCores (1024 tokens each);
replicate gate/sim/expert weights. Each core runs the full routed-MoE
forward for its token shard; host concatenates shard outputs.

Per-core dataflow (tokens-on-partitions for routing / [i|c, t] transposed
layout for the expert matmuls):
  - transpose x via PE into xT [c-chunk(128) x t] tiles
  - logits = (x @ smn) * rsqrt(sum x^2) via PE fp32 + ACT copy-scale
  - routing: relu/STE/inactive detection on DVE; top-16 fallback mask via
    rank matmul (logits @ A, A[e',(a,b)] = d(e'=a)-d(e'=b)), compare+reduce
  - softmax over active entries (exp on ACT, mask/sum/reciprocal on DVE)
  - experts (e, token-block): hT = W1[e].T @ xT (PE fp32r), gelu (ACT),
    y = hT-slice.T @ W2[e] -> [t,c] PSUM (PE fp32r), feo = y * mask col
    (ACT copy-scale), final-acc += feo * rw col (DVE scalar_tensor_tensor),
    DMA feo out.
"""

import os
import sys

if "/opt/trn_rl_repo" not in sys.path:
    sys.path.insert(0, "/opt/trn_rl_repo")

import numpy as np

import concourse.bass as bass
import concourse.bacc as bacc
import concourse.mybir as mybir
import concourse.tile as tile
from concourse import bass_utils
from concourse.masks import make_identity

T, C, I, E = 8192, 512, 64, 32
N_CORES = 8
F32 = mybir.dt.float32
F32R = mybir.dt.float32r
AF = mybir.ActivationFunctionType
ALU = mybir.AluOpType
AX = mybir.AxisListType
KC = C // 128  # contraction chunks of 128


def emit_moe(tc, outs, ins, T_shard, use_f32r=True):
    nc = tc.nc
    x, sim, gates, W1, W2 = ins["x"], ins["sim"], ins["gates"], ins["W1"], ins["W2"]
    feo_o, final_o = outs["feo"], outs["final"]
    pa_o, am_o = outs["pre_act"], outs["am"]

    NTC = T_shard // 128          # 128-token chunks
    TB = min(512, T_shard)        # token block for mm1 free dim
    NTB = T_shard // TB
    TCB = TB // 128               # t-chunks per block

    RD = F32R if use_f32r else F32  # dtype for expert-matmul operands

    with (
        tc.tile_pool(name="persist", bufs=1) as P,
        tc.tile_pool(name="wpool", bufs=1) as WP,
    ):
        # ---- weights: DMA f32, then round to f32r via compute copy ---------
        w1t = [
            [WP.tile([128, I], RD, tag=f"w1_{e}_{k}", name=f"w1_{e}_{k}") for k in range(KC)]
            for e in range(E)
        ]
        w2t = [WP.tile([I, C], RD, tag=f"w2_{e}", name=f"w2_{e}") for e in range(E)]
        def rdcast(ap):
            return ap.bitcast(F32R) if use_f32r else ap

        for e in range(E):
            for k in range(KC):
                nc.sync.dma_start(out=w1t[e][k], in_=rdcast(W1[e, k * 128 : (k + 1) * 128, :]))
            nc.sync.dma_start(out=w2t[e], in_=rdcast(W2[e, :, :]))

        ident = P.tile([128, 128], F32, tag="ident", name="ident")
        make_identity(nc, ident)
        ones_col = P.tile([128, 1], F32, tag="ones_col", name="ones_col")
        nc.vector.memset(ones_col, 1.0)
        ones_row = P.tile([1, 128], F32, tag="ones_row", name="ones_row")
        nc.vector.memset(ones_row, 1.0)

        # sigmoid(gates) broadcast to all partitions
        gsb = P.tile([128, E], F32, tag="gsb", name="gsb")
        g_b = bass.AP(tensor=gates.tensor, offset=gates.offset, ap=[[0, 128]] + list(gates.ap[1:]))
        nc.sync.dma_start(out=gsb, in_=g_b)
        sgb = P.tile([128, E], F32, tag="sgb", name="sgb")
        nc.scalar.activation(sgb, gsb, AF.Sigmoid)

        # ---- sim_matrix l2-normalize along C (on device) -------------------
        simc = [P.tile([128, E], F32, tag=f"simc{k}", name=f"simc{k}") for k in range(KC)]
        for k in range(KC):
            nc.sync.dma_start(out=simc[k], in_=sim[k * 128 : (k + 1) * 128, :])
        smnc = [P.tile([128, E], F32, tag=f"smnc{k}", name=f"smnc{k}") for k in range(KC)]
        with (
            tc.tile_pool(name="pre_t", bufs=2) as T0,
            tc.tile_pool(name="pre_p", bufs=1, space="PSUM") as P0,
        ):
            ss_ps = P0.tile([1, E], F32, tag="ss_ps", name="ss_ps")
            for k in range(KC):
                sim2 = T0.tile([128, E], F32, tag="sim2", name="sim2")
                nc.vector.tensor_mul(sim2, simc[k], simc[k])
                nc.tensor.matmul(ss_ps, lhsT=ones_col, rhs=sim2, start=(k == 0), stop=(k == KC - 1))
            ssc = T0.tile([1, E], F32, tag="ssc", name="ssc")
            nc.vector.tensor_scalar_max(ssc, ss_ps, 1e-24)
            s_sq = T0.tile([1, E], F32, tag="s_sq", name="s_sq")
            nc.scalar.activation(s_sq, ssc, AF.Sqrt)
            rs = T0.tile([1, E], F32, tag="rs", name="rs")
            nc.vector.reciprocal(rs, s_sq)
            rsb_ps = P0.tile([128, E], F32, tag="rsb_ps", name="rsb_ps")
            nc.tensor.matmul(rsb_ps, lhsT=ones_row, rhs=rs, start=True, stop=True)
            rsb = P.tile([128, E], F32, tag="rsb", name="rsb")
            nc.vector.tensor_copy(rsb, rsb_ps)
            for k in range(KC):
                nc.vector.tensor_mul(smnc[k], simc[k], rsb)

        # ---- rank matrix A[e', (a,b)] = d(e'=a) - d(e'=b), [E, E*E] --------
        Amat = P.tile([E, E * E], F32, tag="Amat", name="Amat")
        a3_out = Amat.rearrange("p (a b) -> p a b", a=E)
        i32 = ident[0:E, 0:E]
        in_da = bass.AP(tensor=i32.tensor, offset=i32.offset, ap=[i32.ap[0], i32.ap[1], [0, E]])
        in_db = bass.AP(tensor=i32.tensor, offset=i32.offset, ap=[i32.ap[0], [0, E], i32.ap[1]])
        nc.vector.tensor_sub(a3_out, in_da, in_db)

        # ---- persistent per-token state ------------------------------------
        # xT: exact f32 copy (logits/rank path) + f32r-rounded copy (experts)
        xT = [P.tile([128, T_shard], F32, tag=f"xT{k}", name=f"xT{k}") for k in range(KC)]
        if use_f32r:
            xTr = [P.tile([128, T_shard], RD, tag=f"xTr{k}", name=f"xTr{k}") for k in range(KC)]
        else:
            xTr = xT
        rno_all = P.tile([128, NTC], F32, tag="rno_all", name="rno_all")
        am_all = P.tile([128, NTC * E], F32, tag="am_all", name="am_all")
        rw_all = P.tile([128, NTC * E], F32, tag="rw_all", name="rw_all")
        acc = [P.tile([128, C], F32, tag=f"acc{t}", name=f"acc{t}") for t in range(NTC)]
        for t in range(NTC):
            nc.gpsimd.memset(acc[t], 0.0)

        # ---- phase 1: load x, norms, transpose -----------------------------
        with (
            tc.tile_pool(name="ph1", bufs=3) as T1,
            tc.tile_pool(name="ph1p", bufs=4, space="PSUM") as TP1,
        ):
            for tcid in range(NTC):
                tsl = slice(tcid * 128, (tcid + 1) * 128)
                xn = T1.tile([128, C], F32, tag="xn", name="xn")
                nc.sync.dma_start(out=xn, in_=x[tsl, :])
                scr = T1.tile([128, C], F32, tag="scr", name="scr")
                ssx = T1.tile([128, 1], F32, tag="ssx", name="ssx")
                nc.scalar.activation(scr, xn, AF.Square, accum_out=ssx)
                nc.vector.tensor_scalar_max(ssx, ssx, 1e-24)
                nrm = T1.tile([128, 1], F32, tag="nrm", name="nrm")
                nc.scalar.activation(nrm, ssx, AF.Sqrt)
                nc.vector.reciprocal(rno_all[:, tcid : tcid + 1], nrm)
                for k in range(KC):
                    tps = TP1.tile([128, 128], F32, tag="tps", name="tps")
                    nc.tensor.transpose(tps, xn[:, k * 128 : (k + 1) * 128], ident)
                    nc.vector.tensor_copy(xT[k][:, tsl], tps)
                    if use_f32r:
                        nc.vector.tensor_copy(xTr[k][:, tsl], tps)

        # ---- phase 2: routing ---------------------------------------------
        with (
            tc.tile_pool(name="ph2", bufs=3) as T2,
            tc.tile_pool(name="ph2p", bufs=1, space="PSUM") as TP2,
        ):
            for tcid in range(NTC):
                tsl = slice(tcid * 128, (tcid + 1) * 128)
                esl = slice(tcid * E, (tcid + 1) * E)
                lg_ps = TP2.tile([128, E], F32, tag="lg_ps", name="lg_ps")
                for k in range(KC):
                    nc.tensor.matmul(lg_ps, lhsT=xT[k][:, tsl], rhs=smnc[k], start=(k == 0), stop=(k == KC - 1))
                lg = T2.tile([128, E], F32, tag="lg", name="lg")
                nc.scalar.activation(lg, lg_ps, AF.Copy, scale=rno_all[:, tcid : tcid + 1])
                pa = T2.tile([128, E], F32, tag="pa", name="pa")
                nc.vector.tensor_sub(pa, lg, sgb)
                nc.sync.dma_start(out=pa_o[tsl, :], in_=pa)
                gt = T2.tile([128, E], F32, tag="gt", name="gt")
                nc.vector.tensor_scalar_max(gt, pa, 0.0)
                st = T2.tile([128, E], F32, tag="st", name="st")
                nc.vector.tensor_scalar(st, in0=gt, scalar1=0.0, scalar2=None, op0=ALU.is_gt)
                nact = T2.tile([128, 1], F32, tag="nact", name="nact")
                nc.vector.reduce_sum(nact, st, axis=AX.X)
                ia = T2.tile([128, 1], F32, tag="ia", name="ia")
                nc.vector.tensor_scalar(ia, in0=nact, scalar1=0.0, scalar2=None, op0=ALU.is_equal)
                # top-(E/2) mask of logits via rank counting
                lgT_ps = TP2.tile([E, 128], F32, tag="lgT_ps", name="lgT_ps")
                nc.tensor.transpose(lgT_ps, lg, ident)
                lgT = T2.tile([E, 128], F32, tag="lgT", name="lgT")
                nc.vector.tensor_copy(lgT, lgT_ps)
                D_ps = TP2.tile([128, E * E], F32, tag="D_ps", name="D_ps")
                nc.tensor.matmul(D_ps[:, 0:512], lhsT=lgT, rhs=Amat[:, 0:512], start=True, stop=True)
                nc.tensor.matmul(D_ps[:, 512:1024], lhsT=lgT, rhs=Amat[:, 512:1024], start=True, stop=True)
                Dg = T2.tile([128, E * E], F32, tag="Dg", name="Dg")
                nc.vector.tensor_scalar(Dg, in0=D_ps, scalar1=0.0, scalar2=None, op0=ALU.is_gt)
                rank = T2.tile([128, E], F32, tag="rank", name="rank")
                nc.vector.reduce_sum(rank, Dg.rearrange("p (a b) -> p a b", a=E), axis=AX.X)
                fb = T2.tile([128, E], F32, tag="fb", name="fb")
                nc.vector.tensor_scalar(fb, in0=rank, scalar1=float(E // 2), scalar2=None, op0=ALU.is_ge)
                # activation_mask = ste + inactive * (fb - ste)
                dfs = T2.tile([128, E], F32, tag="dfs", name="dfs")
                nc.vector.tensor_sub(dfs, fb, st)
                amt = am_all[:, esl]
                nc.vector.scalar_tensor_tensor(out=amt, in0=dfs, scalar=ia, in1=st, op0=ALU.mult, op1=ALU.add)
                nc.sync.dma_start(out=am_o[tsl, :], in_=amt)
                # routing weights: softmax over active of gated
                gm = T2.tile([128, E], F32, tag="gm", name="gm")
                nc.vector.tensor_mul(gm, gt, amt)
                negM = T2.tile([128, 1], F32, tag="negM", name="negM")
                nc.vector.reduce_max(negM, gm, axis=AX.X, negate=True)
                ex = T2.tile([128, E], F32, tag="ex", name="ex")
                nc.scalar.activation(ex, gt, AF.Exp, bias=negM)
                nn = T2.tile([128, E], F32, tag="nn", name="nn")
                nc.vector.tensor_mul(nn, ex, amt)
                den = T2.tile([128, 1], F32, tag="den", name="den")
                nc.vector.reduce_sum(den, nn, axis=AX.X)
                rec = T2.tile([128, 1], F32, tag="rec", name="rec")
                nc.vector.reciprocal(rec, den)
                nc.vector.tensor_scalar_mul(rw_all[:, esl], nn, rec)

        # ---- phase 3: experts ---------------------------------------------
        with (
            tc.tile_pool(name="eh", bufs=3) as EH,
            tc.tile_pool(name="efeo", bufs=6) as EF,
            tc.tile_pool(name="ehp", bufs=2, space="PSUM") as EHP,
            tc.tile_pool(name="eyp", bufs=4, space="PSUM") as EYP,
        ):
            for e in range(E):
                for tb in range(NTB):
                    bsl = slice(tb * TB, (tb + 1) * TB)
                    h_ps = EHP.tile([I, TB], F32, tag="h_ps", name="h_ps")
                    for k in range(KC):
                        nc.tensor.matmul(
                            h_ps, lhsT=w1t[e][k], rhs=xTr[k][:, bsl],
                            start=(k == 0), stop=(k == KC - 1),
                        )
                    hs = EH.tile([I, TB], RD, tag="hs", name="hs")
                    nc.scalar.activation(hs, h_ps, AF.Gelu)
                    for tci in range(TCB):
                        tcid = tb * TCB + tci
                        tsl = slice(tcid * 128, (tcid + 1) * 128)
                        col = slice(tcid * E + e, tcid * E + e + 1)
                        y_ps = EYP.tile([128, C], F32, tag="y_ps", name="y_ps")
                        nc.tensor.matmul(
                            y_ps, lhsT=hs[:, tci * 128 : (tci + 1) * 128], rhs=w2t[e],
                            start=True, stop=True,
                        )
                        fe = EF.tile([128, C], F32, tag="fe", name="fe")
                        nc.scalar.activation(fe, y_ps, AF.Copy, scale=am_all[:, col])
                        nc.vector.scalar_tensor_tensor(
                            out=acc[tcid], in0=fe, scalar=rw_all[:, col], in1=acc[tcid],
                            op0=ALU.mult, op1=ALU.add,
                        )
                        nc.sync.dma_start(out=feo_o[tsl, e, :], in_=fe)

        for t in range(NTC):
            nc.sync.dma_start(out=final_o[t * 128 : (t + 1) * 128, :], in_=acc[t])


def build(T_shard=T // N_CORES, use_f32r=True):
    nc = bacc.Bacc("TRN2", debug=False)
    x = nc.dram_tensor("x", [T_shard, C], F32, kind="ExternalInput").ap()
    sim = nc.dram_tensor("sim", [C, E], F32, kind="ExternalInput").ap()
    gates = nc.dram_tensor("gates", [1, E], F32, kind="ExternalInput").ap()
    W1 = nc.dram_tensor("W1", [E, C, I], F32, kind="ExternalInput").ap()
    W2 = nc.dram_tensor("W2", [E, I, C], F32, kind="ExternalInput").ap()
    feo = nc.dram_tensor("feo", [T_shard, E, C], F32, kind="ExternalOutput").ap()
    final = nc.dram_tensor("final", [T_shard, C], F32, kind="ExternalOutput").ap()
    pa = nc.dram_tensor("pre_act", [T_shard, E], F32, kind="ExternalOutput").ap()
    am = nc.dram_tensor("am", [T_shard, E], F32, kind="ExternalOutput").ap()
    with tile.TileContext(nc) as tc:
        emit_moe(
            tc,
            {"feo": feo, "final": final, "pre_act": pa, "am": am},
            {"x": x, "sim": sim, "gates": gates, "W1": W1, "W2": W2},
            T_shard,
            use_f32r,
        )
    nc.compile()
    return nc


def kernel(hidden_states, sim_matrix, gates, W1, W2):
    T_shard = T // N_CORES
    nc = build(T_shard)
    base = {
        "sim": np.ascontiguousarray(np.asarray(sim_matrix, dtype=np.float32)),
        "gates": np.ascontiguousarray(np.asarray(gates, dtype=np.float32).reshape(1, E)),
        "W1": np.ascontiguousarray(np.asarray(W1, dtype=np.float32)),
        "W2": np.ascontiguousarray(np.asarray(W2, dtype=np.float32)),
    }
    hs = np.asarray(hidden_states, dtype=np.float32)
    in_maps = [
        dict(base, x=np.ascontiguousarray(hs[i * T_shard : (i + 1) * T_shard]))
        for i in range(N_CORES)
    ]
    trace = os.environ.get("MOE_TRACE") == "1"
    res = bass_utils.run_bass_kernel_spmd(nc, in_maps, list(range(N_CORES)), trace=trace)
    kernel.last_results = res
    outs = res.results
    final = np.concatenate([o["final"] for o in outs], axis=0)
    feo = np.concatenate([o["feo"] for o in outs], axis=0)
    pa = np.concatenate([o["pre_act"] for o in outs], axis=0)
    am = np.concatenate([o["am"] for o in outs], axis=0)
    return final, feo, pa, am
